# revision 18
# baseline (speedup 1.0000x reference)
"""Trainium2 Bass kernel for nn_Attention_3375844294750.

Cross-attention (q from x, k/v from context) with key mask, 8 heads, d=64.
  B=4, N=M=2048, query_dim=context_dim=512, inner=512.

Sharding: 8 NeuronCores = (batch b = core//2) x (query-half = core%2).
Each core computes attention for its 1024 queries over its batch's keys.
No collectives needed (outputs are disjoint).

Key compaction: masked keys contribute exactly 0 to masked softmax, so the
CPU glue gathers only the unmasked keys (~50% of 2048) per batch, padded
to a multiple of 128; padding slots are killed by the exp bias. This
halves the score/exp/PV work.

Structure: the attention inner loop is ACT(exp)-bound and perfectly
pipelined; everything else (q/k projection tail, output projection,
softmax normalization) is scheduled to run in the PE/DVE idle gaps under
that exp stream.

Per-core math (all matmuls bf16 with fp32 PSUM accumulation):
  qT = (x @ Wq)^T        [inner, n]   via rhs = x^T (CPU pre-transposed)
  kT = (ctx_c @ Wk)^T    [inner, m_c]
  v  = ctx_c @ Wv        [m_c, inner] (+ ones column per head)
  S^T = kT_h-blocks @ qT_h            [m_c, n] per head-pair, K=64
                                      row-tiles run concurrently on PE
  P^T = exp(S*scale + pad_bias)       one-pass softmax (logits bounded,
                                      no max subtraction needed)
  O^T_h (+denom row) = [V_h|1]^T @ P^T_h   accumulated over m-tiles
  O_norm^T = O^T * (1/denom)   (raw-copy to SBUF releases PSUM fast;
                                recip + partition-broadcast via DRAM
                                bounce runs in the background)
  out = O_norm^T-blocks^T @ Wo + bo   (SBUF-accumulated per head-pair)
"""
import os
import sys

for _p in ("/opt/trn_rl_repo", "/root/.axon_site/_ro/trn_rl_repo"):
    if os.path.isdir(_p) and _p not in sys.path:
        sys.path.insert(0, _p)
        break

import numpy as np
import ml_dtypes

B, N, M = 4, 2048, 2048
QD = 512          # query_dim == context_dim
H, D = 8, 64
INNER = H * D     # 512
SCALE = D ** -0.5
NCORE = N // 2    # queries per core = 1024
P = 128
NBLK = 512        # n-block (one PSUM bank per matmul)
MASK_NEG = -1e30

_CACHE = {}


def _build_nc(nmt):
    """Build + compile the SPMD program for nmt m-tiles (m_pad = 128*nmt)."""
    import concourse.mybir as mybir
    from concourse import bacc
    from concourse.tile import TileContext
    import concourse.bass as bass

    mpad = nmt * P
    dt = mybir.dt
    nc = bacc.Bacc("TRN2", target_bir_lowering=False, debug=False, num_devices=8)

    xT_d = nc.declare_dram_parameter("xT", [QD, NCORE], dt.bfloat16, isOutput=False)
    ctxT_d = nc.declare_dram_parameter("ctxT", [QD, mpad], dt.bfloat16, isOutput=False)
    wq_d = nc.declare_dram_parameter("wq", [QD, INNER], dt.bfloat16, isOutput=False)
    wk_d = nc.declare_dram_parameter("wk", [QD, INNER], dt.bfloat16, isOutput=False)
    wv_d = nc.declare_dram_parameter("wv", [QD, INNER], dt.bfloat16, isOutput=False)
    wo_d = nc.declare_dram_parameter("wo", [INNER, QD], dt.bfloat16, isOutput=False)
    bo_d = nc.declare_dram_parameter("bo", [1, QD], dt.float32, isOutput=False)
    mb_d = nc.declare_dram_parameter("mb", [P, nmt], dt.float32, isOutput=False)
    out_d = nc.declare_dram_parameter("out", [NCORE, QD], dt.float32, isOutput=True)

    f32 = dt.float32
    bf16 = dt.bfloat16
    EXP = mybir.ActivationFunctionType.Exp

    with TileContext(nc) as tc:
        from contextlib import ExitStack

        with ExitStack() as ctx:
            const = ctx.enter_context(tc.tile_pool(name="const", bufs=1))

            # ---- persistent SBUF tensors ----
            wq_t = [const.tile([P, INNER], bf16, tag=f"wq{s}", name=f"wq{s}") for s in range(4)]
            xT_t = [const.tile([P, NCORE], bf16, tag=f"xT{s}", name=f"xT{s}") for s in range(4)]
            wk_t = [const.tile([P, INNER], bf16, tag=f"wk{s}", name=f"wk{s}") for s in range(4)]
            ctxT_t = [const.tile([P, mpad], bf16, tag=f"cT{s}", name=f"cT{s}") for s in range(4)]
            wv_sb = const.tile([P, 4, INNER], bf16, tag="wv")
            wo_sb = const.tile([P, 4, QD], bf16, tag="wo")
            bo_bc = const.tile([P, QD], f32, tag="bo")
            mb_sb = const.tile([P, nmt], f32, tag="mb")

            qT_sb = const.tile([P, 4, NCORE], bf16, tag="qT")
            kT_sb = const.tile([P, 4, mpad], bf16, tag="kT")
            v_sb = const.tile([P, nmt, H, D + 1], bf16, tag="v")
            o_sb = const.tile([P, 4, NCORE], bf16, tag="oT")
            fin_sb = const.tile([P, NCORE // P, QD], f32, tag="fin")

            # ---- input loads, interleaved so the first matmuls start early
            for s in range(4):
                nc.sync.dma_start(out=wq_t[s][:], in_=wq_d[s * P:(s + 1) * P, :])
                nc.sync.dma_start(out=xT_t[s][:], in_=xT_d[s * P:(s + 1) * P, :])
            for s in range(4):
                nc.sync.dma_start(out=wk_t[s][:], in_=wk_d[s * P:(s + 1) * P, :])
                nc.sync.dma_start(out=ctxT_t[s][:], in_=ctxT_d[s * P:(s + 1) * P, :])
            for s in range(4):
                nc.sync.dma_start(out=wv_sb[:, s, :], in_=wv_d[s * P:(s + 1) * P, :])
                nc.sync.dma_start(out=wo_sb[:, s, :], in_=wo_d[s * P:(s + 1) * P, :])
            nc.sync.dma_start(out=mb_sb[:], in_=mb_d[:])
            bo_src = bass.AP(tensor=bo_d.ap().tensor, offset=bo_d.ap().offset,
                             ap=[[0, P]] + bo_d.ap().ap[1:])
            nc.sync.dma_start(out=bo_bc[:], in_=bo_src)

            # ones columns for the denominator trick (copies below leave them)
            nc.vector.memset(v_sb[:], 1.0)

            mchunks = []
            off = 0
            while off < mpad:
                w = min(NBLK, mpad - off)
                mchunks.append((off, w))
                off += w

            with tc.tile_pool(name="aux", bufs=2, space="PSUM") as aux, \
                 tc.tile_pool(name="sps", bufs=2, space="PSUM") as sps, \
                 tc.tile_pool(name="ops", bufs=1, space="PSUM") as ops, \
                 tc.tile_pool(name="ppool", bufs=4) as ppool, \
                 tc.tile_pool(name="raw", bufs=4) as rawp, \
                 tc.tile_pool(name="nrm", bufs=2) as nrmp, \
                 tc.tile_pool(name="dscr", bufs=2, space="DRAM") as dscr:

                def proj_q_k(mi):
                    # qT [inner, n] slice mi
                    for nh in range(2):
                        ps = aux.tile([P, NBLK], f32, tag="aux")
                        for kq in range(4):
                            nc.tensor.matmul(
                                ps[:],
                                lhsT=wq_t[kq][:, mi * P:(mi + 1) * P],
                                rhs=xT_t[kq][:, nh * NBLK:(nh + 1) * NBLK],
                                start=(kq == 0), stop=(kq == 3),
                            )
                        nc.vector.tensor_copy(
                            qT_sb[:, mi, nh * NBLK:(nh + 1) * NBLK], ps[:])
                    # kT [inner, m_pad] slice mi
                    for off, w in mchunks:
                        ps = aux.tile([P, NBLK], f32, tag="aux")
                        for kq in range(4):
                            nc.tensor.matmul(
                                ps[:, 0:w],
                                lhsT=wk_t[kq][:, mi * P:(mi + 1) * P],
                                rhs=ctxT_t[kq][:, off:off + w],
                                start=(kq == 0), stop=(kq == 3),
                            )
                        nc.vector.tensor_copy(
                            kT_sb[:, mi, off:off + w], ps[:, 0:w])

                # pair 0 needs slice 0 of qT/kT and all of v before attention
                proj_q_k(0)
                for mt in range(nmt):
                    ps = aux.tile([P, INNER], f32, tag="aux")
                    for kq in range(4):
                        nc.tensor.matmul(
                            ps[:],
                            lhsT=ctxT_t[kq][:, mt * P:(mt + 1) * P],
                            rhs=wv_sb[:, kq, :],
                            start=(kq == 0), stop=(kq == 3),
                        )
                    psh = ps.rearrange("p (h d) -> p h d", h=H)
                    nc.vector.tensor_copy(v_sb[:, mt, :, 0:D], psh[:])

                # ---- attention, one head-pair (2p, 2p+1) at a time ----
                for p in range(4):
                    hA, hB = 2 * p, 2 * p + 1
                    rawa = rawp.tile([P, NCORE], f32, tag="rawa")
                    rawb = rawp.tile([P, NCORE], f32, tag="rawb")
                    for nb in range(2):
                        nsl = slice(nb * NBLK, (nb + 1) * NBLK)
                        oa = ops.tile([P, NBLK], f32, tag="oa")
                        ob = ops.tile([P, NBLK], f32, tag="ob")
                        for mt in range(nmt):
                            sp = sps.tile([P, 2 * NBLK], f32, tag="s")
                            msl = slice(mt * P, (mt + 1) * P)
                            nc.tensor.matmul(
                                sp[:, 0:NBLK],
                                lhsT=kT_sb[0:64, p, msl],
                                rhs=qT_sb[0:64, p, nsl],
                                start=True, stop=True,
                            )
                            nc.tensor.matmul(
                                sp[:, NBLK:2 * NBLK],
                                lhsT=kT_sb[64:128, p, msl],
                                rhs=qT_sb[64:128, p, nsl],
                                start=True, stop=True,
                            )
                            pt = ppool.tile([P, 2 * NBLK], bf16, tag="pt")
                            nc.scalar.activation(
                                out=pt[:], in_=sp[:], func=EXP,
                                bias=mb_sb[:, mt:mt + 1], scale=SCALE,
                            )
                            nc.tensor.matmul(
                                oa[0:D + 1, :],
                                lhsT=v_sb[:, mt, hA, :],
                                rhs=pt[:, 0:NBLK],
                                start=(mt == 0), stop=(mt == nmt - 1),
                            )
                            nc.tensor.matmul(
                                ob[0:D + 1, :],
                                lhsT=v_sb[:, mt, hB, :],
                                rhs=pt[:, NBLK:2 * NBLK],
                                start=(mt == 0), stop=(mt == nmt - 1),
                            )
                        # fast copies release the PSUM accumulators
                        nc.vector.tensor_copy(rawa[0:D + 1, nsl],
                                              oa[0:D + 1, :])
                        nc.vector.tensor_copy(rawb[0:D + 1, nsl],
                                              ob[0:D + 1, :])

                    # interleave next pair's q/k projection under this
                    # pair's tail
                    if p < 3:
                        proj_q_k(p + 1)

                    # background normalization chain
                    rcb = nrmp.tile([64, 2, NCORE], f32, tag="rcb")
                    bcb = nrmp.tile([64, 2, NCORE], f32, tag="bcb")
                    scr = dscr.tile([2, NCORE], f32, tag="scr")
                    for i, raw in ((0, rawa), (1, rawb)):
                        nc.sync.dma_start(out=scr[i:i + 1, :],
                                          in_=raw[64:65, :])
                        src = scr[i:i + 1, :]
                        bsrc = bass.AP(tensor=src.tensor, offset=src.offset,
                                       ap=[[0, 64]] + src.ap[1:])
                        nc.sync.dma_start(out=rcb[0:64, i, :], in_=bsrc)
                    nc.vector.reciprocal_approx_fast(
                        out=bcb[0:64, :, :], in_=rcb[0:64, :, :])
                    nc.vector.tensor_mul(
                        o_sb[0:64, p, :], rawa[0:64, :], bcb[0:64, 0, :])
                    tb = nrmp.tile([64, NCORE], bf16, tag="tb")
                    nc.vector.tensor_mul(
                        tb[0:64, :], rawb[0:64, :], bcb[0:64, 1, :])
                    nc.sync.dma_start(out=o_sb[64:128, p, :], in_=tb[0:64, :])

                    # output projection contribution of this pair,
                    # accumulated in SBUF under the next pair's exp stream
                    for nt in range(NCORE // P):
                        ps = aux.tile([P, NBLK], f32, tag="aux")
                        nc.tensor.matmul(
                            ps[:, 0:QD],
                            lhsT=o_sb[:, p, nt * P:(nt + 1) * P],
                            rhs=wo_sb[:, p, :],
                            start=True, stop=True,
                        )
                        if p == 0:
                            nc.vector.tensor_add(
                                fin_sb[:, nt, :], ps[:, 0:QD], bo_bc[:])
                        else:
                            nc.vector.tensor_add(
                                fin_sb[:, nt, :], ps[:, 0:QD],
                                fin_sb[:, nt, :])
                        if p == 3:
                            nc.sync.dma_start(
                                out=out_d[nt * P:(nt + 1) * P, :],
                                in_=fin_sb[:, nt, :])

    nc.compile()
    return nc


def get_nc(nmt=None):
    if nmt is None:
        nmt = _CACHE.get("last_nmt", M // P)
    if ("nc", nmt) not in _CACHE:
        _CACHE[("nc", nmt)] = _build_nc(nmt)
    _CACHE["last_nmt"] = nmt
    return _CACHE[("nc", nmt)]


def make_in_maps(x, context, mask, Wq, Wkv, Wo, bo):
    """CPU glue: shard, transpose, cast, and compact keys by mask."""
    bf = ml_dtypes.bfloat16
    Wk = np.ascontiguousarray(Wkv[:, :INNER]).astype(bf)
    Wv = np.ascontiguousarray(Wkv[:, INNER:]).astype(bf)
    Wq_b = np.ascontiguousarray(Wq).astype(bf)
    Wo_b = np.ascontiguousarray(Wo).astype(bf)
    bo_f = np.ascontiguousarray(bo, dtype=np.float32).reshape(1, QD)

    idxs = [np.where(mask[b])[0] for b in range(B)]
    maxc = max(1, max(len(i) for i in idxs))
    nmt = (maxc + P - 1) // P
    mpad = nmt * P

    in_maps = []
    for c in range(8):
        b, s = c // 2, c % 2
        idx = idxs[b]
        cnt = len(idx)
        ctx_c = np.zeros((mpad, QD), dtype=np.float32)
        ctx_c[:cnt] = context[b][idx]
        mb = np.full(mpad, MASK_NEG, dtype=np.float32)
        mb[:cnt] = 0.0
        xT = np.ascontiguousarray(
            x[b, s * NCORE:(s + 1) * NCORE, :].T).astype(bf)
        ctxT = np.ascontiguousarray(ctx_c.T).astype(bf)
        mbt = np.ascontiguousarray(mb.reshape(nmt, P).T)
        in_maps.append({
            "xT": xT, "ctxT": ctxT, "wq": Wq_b, "wk": Wk, "wv": Wv,
            "wo": Wo_b, "bo": bo_f, "mb": mbt,
        })
    return in_maps, nmt


def assemble(results):
    out = np.empty((B, N, QD), dtype=np.float32)
    for c in range(8):
        b, s = c // 2, c % 2
        out[b, s * NCORE:(s + 1) * NCORE, :] = results[c]["out"]
    return out


def kernel(x, context, mask, Wq, Wkv, Wo, bo):
    from concourse.bass_utils import run_bass_kernel_spmd

    x = np.asarray(x, dtype=np.float32)
    context = np.asarray(context, dtype=np.float32)
    mask = np.asarray(mask)
    in_maps, nmt = make_in_maps(x, context, mask,
                                np.asarray(Wq, dtype=np.float32),
                                np.asarray(Wkv, dtype=np.float32),
                                np.asarray(Wo, dtype=np.float32),
                                np.asarray(bo, dtype=np.float32))
    nc = get_nc(nmt)
    res = run_bass_kernel_spmd(nc, in_maps, list(range(8)))
    return assemble(res.results)


# revision 20
# speedup vs baseline: 1.0380x; 1.0380x over previous
"""Trainium2 Bass kernel for nn_Attention_3375844294750.

Cross-attention (q from x, k/v from context) with key mask, 8 heads, d=64.
  B=4, N=M=2048, query_dim=context_dim=512, inner=512.

Sharding: 8 NeuronCores = (batch b = core//2) x (query-half = core%2).
Each core computes attention for its 1024 queries over its batch's keys.
No collectives needed (outputs are disjoint).

Key compaction: masked keys contribute exactly 0 to masked softmax, so the
CPU glue gathers only the unmasked keys (~50% of 2048) per batch, padded
to a multiple of 128; padding slots are killed by the exp bias. This
halves the score/exp/PV work.

Structure: the attention inner loop is ACT(exp)-bound and perfectly
pipelined; everything else (q/k projection tail, output projection,
softmax normalization) is scheduled to run in the PE/DVE idle gaps under
that exp stream.

Per-core math (all matmuls bf16 with fp32 PSUM accumulation):
  qT = (x @ Wq)^T        [inner, n]   via rhs = x^T (CPU pre-transposed)
  kT = (ctx_c @ Wk)^T    [inner, m_c]
  v  = ctx_c @ Wv        [m_c, inner] (+ ones column per head)
  S^T = kT_h-blocks @ qT_h            [m_c, n] per head-pair, K=64
                                      row-tiles run concurrently on PE
  P^T = exp(S*scale + pad_bias)       one-pass softmax (logits bounded,
                                      no max subtraction needed)
  O^T_h (+denom row) = [V_h|1]^T @ P^T_h   accumulated over m-tiles
  O_norm^T = O^T * (1/denom)   (raw-copy to SBUF releases PSUM fast;
                                recip + partition-broadcast via DRAM
                                bounce runs in the background)
  out = O_norm^T-blocks^T @ Wo + bo   (SBUF-accumulated per head-pair)
"""
import os
import sys

for _p in ("/opt/trn_rl_repo", "/root/.axon_site/_ro/trn_rl_repo"):
    if os.path.isdir(_p) and _p not in sys.path:
        sys.path.insert(0, _p)
        break

import numpy as np
import ml_dtypes

B, N, M = 4, 2048, 2048
QD = 512          # query_dim == context_dim
H, D = 8, 64
INNER = H * D     # 512
SCALE = D ** -0.5
NCORE = N // 2    # queries per core = 1024
P = 128
NBLK = 512        # n-block (one PSUM bank per matmul)
MASK_NEG = -1e30

_CACHE = {}


def _build_nc(nmt):
    """Build + compile the SPMD program for nmt m-tiles (m_pad = 128*nmt)."""
    import concourse.mybir as mybir
    from concourse import bacc
    from concourse.tile import TileContext
    import concourse.bass as bass

    mpad = nmt * P
    dt = mybir.dt
    nc = bacc.Bacc("TRN2", target_bir_lowering=False, debug=False, num_devices=8)

    xT_d = nc.declare_dram_parameter("xT", [QD, NCORE], dt.bfloat16, isOutput=False)
    ctxT_d = nc.declare_dram_parameter("ctxT", [QD, mpad], dt.bfloat16, isOutput=False)
    wq_d = nc.declare_dram_parameter("wq", [QD, INNER], dt.bfloat16, isOutput=False)
    wk_d = nc.declare_dram_parameter("wk", [QD, INNER], dt.bfloat16, isOutput=False)
    wv_d = nc.declare_dram_parameter("wv", [QD, INNER], dt.bfloat16, isOutput=False)
    wo_d = nc.declare_dram_parameter("wo", [INNER, QD], dt.bfloat16, isOutput=False)
    bo_d = nc.declare_dram_parameter("bo", [1, QD], dt.float32, isOutput=False)
    mb_d = nc.declare_dram_parameter("mb", [P, nmt], dt.float32, isOutput=False)
    out_d = nc.declare_dram_parameter("out", [NCORE, QD], dt.float32, isOutput=True)

    f32 = dt.float32
    bf16 = dt.bfloat16
    EXP = mybir.ActivationFunctionType.Exp

    with TileContext(nc) as tc:
        from contextlib import ExitStack

        with ExitStack() as ctx:
            const = ctx.enter_context(tc.tile_pool(name="const", bufs=1))

            # ---- persistent SBUF tensors ----
            wq_t = [const.tile([P, INNER], bf16, tag=f"wq{s}", name=f"wq{s}") for s in range(4)]
            xT_t = [const.tile([P, NCORE], bf16, tag=f"xT{s}", name=f"xT{s}") for s in range(4)]
            wk_t = [const.tile([P, INNER], bf16, tag=f"wk{s}", name=f"wk{s}") for s in range(4)]
            ctxT_t = [const.tile([P, mpad], bf16, tag=f"cT{s}", name=f"cT{s}") for s in range(4)]
            wv_sb = const.tile([P, 4, INNER], bf16, tag="wv")
            wo_sb = const.tile([P, 4, QD], bf16, tag="wo")
            bo_bc = const.tile([P, QD], f32, tag="bo")
            mb_sb = const.tile([P, nmt], f32, tag="mb")

            qT_sb = const.tile([P, 4, NCORE], bf16, tag="qT")
            kT_sb = const.tile([P, 4, mpad], bf16, tag="kT")
            v_sb = const.tile([P, nmt, H, D + 1], bf16, tag="v")
            o_sb = const.tile([P, 4, NCORE], bf16, tag="oT")
            fin_sb = const.tile([P, NCORE // P, QD], f32, tag="fin")

            # ---- input loads, interleaved so the first matmuls start early
            for s in range(4):
                nc.sync.dma_start(out=wq_t[s][:], in_=wq_d[s * P:(s + 1) * P, :])
                nc.sync.dma_start(out=xT_t[s][:], in_=xT_d[s * P:(s + 1) * P, :])
            for s in range(4):
                nc.sync.dma_start(out=wk_t[s][:], in_=wk_d[s * P:(s + 1) * P, :])
                nc.sync.dma_start(out=ctxT_t[s][:], in_=ctxT_d[s * P:(s + 1) * P, :])
            for s in range(4):
                nc.sync.dma_start(out=wv_sb[:, s, :], in_=wv_d[s * P:(s + 1) * P, :])
                nc.sync.dma_start(out=wo_sb[:, s, :], in_=wo_d[s * P:(s + 1) * P, :])
            nc.sync.dma_start(out=mb_sb[:], in_=mb_d[:])
            bo_src = bass.AP(tensor=bo_d.ap().tensor, offset=bo_d.ap().offset,
                             ap=[[0, P]] + bo_d.ap().ap[1:])
            nc.sync.dma_start(out=bo_bc[:], in_=bo_src)

            # ones columns for the denominator trick (copies below leave them)
            nc.vector.memset(v_sb[:], 1.0)

            mchunks = []
            off = 0
            while off < mpad:
                w = min(NBLK, mpad - off)
                mchunks.append((off, w))
                off += w

            with tc.tile_pool(name="aux", bufs=2, space="PSUM") as aux, \
                 tc.tile_pool(name="sps", bufs=2, space="PSUM") as sps, \
                 tc.tile_pool(name="ops", bufs=1, space="PSUM") as ops, \
                 tc.tile_pool(name="ppool", bufs=4) as ppool, \
                 tc.tile_pool(name="raw", bufs=4) as rawp, \
                 tc.tile_pool(name="nrm", bufs=2) as nrmp, \
                 tc.tile_pool(name="dscr", bufs=2, space="DRAM") as dscr:

                def proj_q_k(mi):
                    # qT [inner, n] slice mi
                    for nh in range(2):
                        ps = aux.tile([P, NBLK], f32, tag="aux")
                        for kq in range(4):
                            nc.tensor.matmul(
                                ps[:],
                                lhsT=wq_t[kq][:, mi * P:(mi + 1) * P],
                                rhs=xT_t[kq][:, nh * NBLK:(nh + 1) * NBLK],
                                start=(kq == 0), stop=(kq == 3),
                            )
                        nc.vector.tensor_copy(
                            qT_sb[:, mi, nh * NBLK:(nh + 1) * NBLK], ps[:])
                    # kT [inner, m_pad] slice mi
                    for off, w in mchunks:
                        ps = aux.tile([P, NBLK], f32, tag="aux")
                        for kq in range(4):
                            nc.tensor.matmul(
                                ps[:, 0:w],
                                lhsT=wk_t[kq][:, mi * P:(mi + 1) * P],
                                rhs=ctxT_t[kq][:, off:off + w],
                                start=(kq == 0), stop=(kq == 3),
                            )
                        nc.vector.tensor_copy(
                            kT_sb[:, mi, off:off + w], ps[:, 0:w])

                # pair 0 needs slice 0 of qT/kT and all of v before attention
                proj_q_k(0)
                for mt in range(nmt):
                    ps = aux.tile([P, INNER], f32, tag="aux")
                    for kq in range(4):
                        nc.tensor.matmul(
                            ps[:],
                            lhsT=ctxT_t[kq][:, mt * P:(mt + 1) * P],
                            rhs=wv_sb[:, kq, :],
                            start=(kq == 0), stop=(kq == 3),
                        )
                    psh = ps.rearrange("p (h d) -> p h d", h=H)
                    nc.vector.tensor_copy(v_sb[:, mt, :, 0:D], psh[:])

                # deferred aux work (projection slices, output-proj
                # units) dripped into the attention stream one unit per
                # iteration so the exp pipeline never starves
                pending = []

                def proj_unit_q(mi, nh):
                    def f():
                        ps = aux.tile([P, NBLK], f32, tag="aux", name="psq")
                        for kq in range(4):
                            nc.tensor.matmul(
                                ps[:],
                                lhsT=wq_t[kq][:, mi * P:(mi + 1) * P],
                                rhs=xT_t[kq][:, nh * NBLK:(nh + 1) * NBLK],
                                start=(kq == 0), stop=(kq == 3),
                            )
                        nc.vector.tensor_copy(
                            qT_sb[:, mi, nh * NBLK:(nh + 1) * NBLK], ps[:])
                    return f

                def proj_unit_k(mi, off, w):
                    def f():
                        ps = aux.tile([P, NBLK], f32, tag="aux", name="psk")
                        for kq in range(4):
                            nc.tensor.matmul(
                                ps[:, 0:w],
                                lhsT=wk_t[kq][:, mi * P:(mi + 1) * P],
                                rhs=ctxT_t[kq][:, off:off + w],
                                start=(kq == 0), stop=(kq == 3),
                            )
                        nc.vector.tensor_copy(
                            kT_sb[:, mi, off:off + w], ps[:, 0:w])
                    return f

                def fin_unit(p, nt):
                    def f():
                        ps = aux.tile([P, NBLK], f32, tag="aux", name="psf")
                        nc.tensor.matmul(
                            ps[:, 0:QD],
                            lhsT=o_sb[:, p, nt * P:(nt + 1) * P],
                            rhs=wo_sb[:, p, :],
                            start=True, stop=True,
                        )
                        if p == 0:
                            nc.vector.tensor_add(
                                fin_sb[:, nt, :], ps[:, 0:QD], bo_bc[:])
                        else:
                            nc.vector.tensor_add(
                                fin_sb[:, nt, :], ps[:, 0:QD],
                                fin_sb[:, nt, :])
                        if p == 3:
                            nc.sync.dma_start(
                                out=out_d[nt * P:(nt + 1) * P, :],
                                in_=fin_sb[:, nt, :])
                    return f

                # ---- attention, one head-pair (2p, 2p+1) at a time ----
                for p in range(4):
                    hA, hB = 2 * p, 2 * p + 1
                    # projection for this pair must be emitted before its
                    # first score matmul: flush any backlog
                    for f in pending:
                        f()
                    pending = []
                    rawa = rawp.tile([P, NCORE], f32, tag="rawa")
                    rawb = rawp.tile([P, NCORE], f32, tag="rawb")
                    if p < 3:
                        for nh in range(2):
                            pending.append(proj_unit_q(p + 1, nh))
                        for off, w in mchunks:
                            pending.append(proj_unit_k(p + 1, off, w))
                    for nb in range(2):
                        nsl = slice(nb * NBLK, (nb + 1) * NBLK)
                        oa = ops.tile([P, NBLK], f32, tag="oa")
                        ob = ops.tile([P, NBLK], f32, tag="ob")
                        for mt in range(nmt):
                            sp = sps.tile([P, 2 * NBLK], f32, tag="s")
                            msl = slice(mt * P, (mt + 1) * P)
                            nc.tensor.matmul(
                                sp[:, 0:NBLK],
                                lhsT=kT_sb[0:64, p, msl],
                                rhs=qT_sb[0:64, p, nsl],
                                start=True, stop=True,
                            )
                            nc.tensor.matmul(
                                sp[:, NBLK:2 * NBLK],
                                lhsT=kT_sb[64:128, p, msl],
                                rhs=qT_sb[64:128, p, nsl],
                                start=True, stop=True,
                            )
                            pt = ppool.tile([P, 2 * NBLK], bf16, tag="pt")
                            nc.scalar.activation(
                                out=pt[:], in_=sp[:], func=EXP,
                                bias=mb_sb[:, mt:mt + 1], scale=SCALE,
                            )
                            nc.tensor.matmul(
                                oa[0:D + 1, :],
                                lhsT=v_sb[:, mt, hA, :],
                                rhs=pt[:, 0:NBLK],
                                start=(mt == 0), stop=(mt == nmt - 1),
                            )
                            nc.tensor.matmul(
                                ob[0:D + 1, :],
                                lhsT=v_sb[:, mt, hB, :],
                                rhs=pt[:, NBLK:2 * NBLK],
                                start=(mt == 0), stop=(mt == nmt - 1),
                            )
                            if pending:
                                pending.pop(0)()
                        # fast copies release the PSUM accumulators
                        nc.vector.tensor_copy(rawa[0:D + 1, nsl],
                                              oa[0:D + 1, :])
                        nc.vector.tensor_copy(rawb[0:D + 1, nsl],
                                              ob[0:D + 1, :])

                        # per-half background normalization chain
                        rcb = nrmp.tile([64, 2, NBLK], f32, tag="rcb")
                        bcb = nrmp.tile([64, 2, NBLK], f32, tag="bcb")
                        scr = dscr.tile([2, NBLK], f32, tag="scr")
                        for i, raw in ((0, rawa), (1, rawb)):
                            nc.sync.dma_start(out=scr[i:i + 1, :],
                                              in_=raw[64:65, nsl])
                            src = scr[i:i + 1, :]
                            bsrc = bass.AP(tensor=src.tensor,
                                           offset=src.offset,
                                           ap=[[0, 64]] + src.ap[1:])
                            nc.sync.dma_start(out=rcb[0:64, i, :], in_=bsrc)
                        nc.vector.reciprocal_approx_fast(
                            out=bcb[0:64, :, :], in_=rcb[0:64, :, :])
                        nc.vector.tensor_mul(
                            o_sb[0:64, p, nsl], rawa[0:64, nsl],
                            bcb[0:64, 0, :])
                        tb = nrmp.tile([64, NBLK], bf16, tag="tb")
                        nc.vector.tensor_mul(
                            tb[0:64, :], rawb[0:64, nsl], bcb[0:64, 1, :])
                        nc.sync.dma_start(out=o_sb[64:128, p, nsl],
                                          in_=tb[0:64, :])
                        for nt in range(nb * 4, nb * 4 + 4):
                            pending.append(fin_unit(p, nt))
                # drain any remaining aux work (last pair's output proj)
                for f in pending:
                    f()

    nc.compile()
    return nc


def get_nc(nmt=None):
    if nmt is None:
        nmt = _CACHE.get("last_nmt", M // P)
    if ("nc", nmt) not in _CACHE:
        _CACHE[("nc", nmt)] = _build_nc(nmt)
    _CACHE["last_nmt"] = nmt
    return _CACHE[("nc", nmt)]


def make_in_maps(x, context, mask, Wq, Wkv, Wo, bo):
    """CPU glue: shard, transpose, cast, and compact keys by mask."""
    bf = ml_dtypes.bfloat16
    Wk = np.ascontiguousarray(Wkv[:, :INNER]).astype(bf)
    Wv = np.ascontiguousarray(Wkv[:, INNER:]).astype(bf)
    Wq_b = np.ascontiguousarray(Wq).astype(bf)
    Wo_b = np.ascontiguousarray(Wo).astype(bf)
    bo_f = np.ascontiguousarray(bo, dtype=np.float32).reshape(1, QD)

    idxs = [np.where(mask[b])[0] for b in range(B)]
    maxc = max(1, max(len(i) for i in idxs))
    nmt = (maxc + P - 1) // P
    mpad = nmt * P

    in_maps = []
    for c in range(8):
        b, s = c // 2, c % 2
        idx = idxs[b]
        cnt = len(idx)
        ctx_c = np.zeros((mpad, QD), dtype=np.float32)
        ctx_c[:cnt] = context[b][idx]
        mb = np.full(mpad, MASK_NEG, dtype=np.float32)
        mb[:cnt] = 0.0
        xT = np.ascontiguousarray(
            x[b, s * NCORE:(s + 1) * NCORE, :].T).astype(bf)
        ctxT = np.ascontiguousarray(ctx_c.T).astype(bf)
        mbt = np.ascontiguousarray(mb.reshape(nmt, P).T)
        in_maps.append({
            "xT": xT, "ctxT": ctxT, "wq": Wq_b, "wk": Wk, "wv": Wv,
            "wo": Wo_b, "bo": bo_f, "mb": mbt,
        })
    return in_maps, nmt


def assemble(results):
    out = np.empty((B, N, QD), dtype=np.float32)
    for c in range(8):
        b, s = c // 2, c % 2
        out[b, s * NCORE:(s + 1) * NCORE, :] = results[c]["out"]
    return out


def kernel(x, context, mask, Wq, Wkv, Wo, bo):
    from concourse.bass_utils import run_bass_kernel_spmd

    x = np.asarray(x, dtype=np.float32)
    context = np.asarray(context, dtype=np.float32)
    mask = np.asarray(mask)
    in_maps, nmt = make_in_maps(x, context, mask,
                                np.asarray(Wq, dtype=np.float32),
                                np.asarray(Wkv, dtype=np.float32),
                                np.asarray(Wo, dtype=np.float32),
                                np.asarray(bo, dtype=np.float32))
    nc = get_nc(nmt)
    res = run_bass_kernel_spmd(nc, in_maps, list(range(8)))
    return assemble(res.results)
